# revision 20
# baseline (speedup 1.0000x reference)
"""Causal self-attention with RoPE on 8 Trainium2 NeuronCores.

Sharding: core c = 4*b + g handles batch b (of 2) and head group g (4 of 16
heads). Each core computes q/k/v projections for its heads, head-local causal
softmax attention, and a partial output projection (Wp columns of its heads);
the host sums the 4 partials per batch.

Layout strategy (per core):
  xT    [C, T]  : x[b] transposed (host) — contraction operand for QKV.
  qT/kT [128,T] : per j-tile (2 heads each), partition = head dim.
  v''   [128,260] x16 : natural layout per t-tile; 65 cols/head =
                  [ones | v_head] so the AV matmul's row 0 accumulates the
                  softmax denominator for free.
  S^T   [s, t]  : scores transposed; exp(0.125*(S+mask)) on ACT -> P^T.
  AV    [65, t] : yT_unnorm (rows 1..64) + r (row 0) per head.
  norm  : 1/r broadcast to [128, t] via a K=4 indicator matmul; 1 DVE mult.
  proj  : outT[e, t] partial = WpT_g.T @ yT  (+ bp on group-leader core).

All matmul operands are float32r (TF32-like rounding, ~12 mantissa bits,
4x faster than fp32 on the PE). Causality is exploited at 128-tile
granularity; diagonal tiles are masked additively before the exp.
"""

import sys

for _p in ("/opt/trn_rl_repo",):
    if _p not in sys.path:
        sys.path.append(_p)

import numpy as np
from contextlib import ExitStack

import concourse.bacc as bacc
import concourse.tile as tile
from concourse import mybir
from concourse.bass_utils import run_bass_kernel_spmd

F32 = mybir.dt.float32
F32R = mybir.dt.float32r
EXP = mybir.ActivationFunctionType.Exp

B, T, C = 2, 2048, 1024
H, D = 16, 64
HG = 4                 # heads per core
JG = HG * D            # 256 j-columns per core
VW = HG * 65           # v'' width (ones col + 64 dims per head)
NKT = C // 128         # 8 contraction tiles
NTT = T // 128         # 16 t-tiles / s-tiles
NC4 = T // 512         # 4 512-chunks
MASK_VAL = -30000.0
SCALE = 1.0 / np.sqrt(D)

_NC_CACHE = None


def build_bass(debug=False, zero_bias=False):
    nc = bacc.Bacc()

    xT = nc.declare_dram_parameter("xT", [C, T], F32, isOutput=False)
    wqT = nc.declare_dram_parameter("wqT", [C, JG], F32, isOutput=False)
    wkT = nc.declare_dram_parameter("wkT", [C, JG], F32, isOutput=False)
    wvT = nc.declare_dram_parameter("wvT", [C, JG], F32, isOutput=False)
    wpT = nc.declare_dram_parameter("wpT", [JG, C], F32, isOutput=False)
    bq_r = nc.declare_dram_parameter("bq_r", [1, JG], F32, isOutput=False)
    bk_r = nc.declare_dram_parameter("bk_r", [1, JG], F32, isOutput=False)
    bv_r = nc.declare_dram_parameter("bv_r", [1, JG], F32, isOutput=False)
    bp_r = nc.declare_dram_parameter("bp_r", [1, C], F32, isOutput=False)
    cosT = nc.declare_dram_parameter("cosT", [128, T], F32, isOutput=False)
    ssT = nc.declare_dram_parameter("ssT", [128, T], F32, isOutput=False)
    bmask = nc.declare_dram_parameter("bmask", [128, 384], F32, isOutput=False)
    ind = nc.declare_dram_parameter("ind", [2, JG], F32, isOutput=False)
    ones_r = nc.declare_dram_parameter("ones_r", [1, 512], F32, isOutput=False)
    vones = nc.declare_dram_parameter("vones", [128, HG], F32, isOutput=False)

    outT = nc.declare_dram_parameter("outT", [C, T], F32, isOutput=True)
    if debug:
        d_qT = [nc.declare_dram_parameter(f"d_qT{j}", [128, T], F32, isOutput=True) for j in range(2)]
        d_kT = [nc.declare_dram_parameter(f"d_kT{j}", [128, T], F32, isOutput=True) for j in range(2)]
        d_yT = [nc.declare_dram_parameter(f"d_yT{j}", [128, T], F32, isOutput=True) for j in range(2)]
        d_v = [nc.declare_dram_parameter(f"d_v{s}", [128, VW], F32, isOutput=True) for s in range(NTT)]
        d_rr4 = nc.declare_dram_parameter("d_rr4", [HG, T], F32, isOutput=True)

    with (
        tile.TileContext(nc) as tc,
        ExitStack() as ctx,
        nc.allow_low_precision(reason="f32r matmul pipeline"),
    ):
        consts = ctx.enter_context(tc.tile_pool(name="consts", bufs=1))

        def load_const(name, dram, shape, dtype=F32R):
            t = consts.tile(shape, dtype, tag=name, name=name)
            src = dram[:] if dtype is F32 else dram[:].bitcast(F32R)
            nc.gpsimd.dma_start(t[:], src)
            return t

        # weights as [128, nk*width]: contraction tile i lives at cols [i*w,(i+1)*w)
        def load_w(name, dram, width, eng=None):
            t = consts.tile([128, NKT * width], F32R, tag=name, name=name)
            (eng or nc.gpsimd).dma_start(
                t[:].rearrange("p (i j) -> p i j", i=NKT),
                dram[:].bitcast(F32R).rearrange("(i p) j -> p i j", p=128),
            )
            return t

        wq_sb = consts.tile([128, NKT * JG], F32R, tag="wq", name="wq")
        # big consts are DMA'd on the sync queue interleaved with the xt
        # stream (see qk loop); small/late consts go via gpsimd SWDGE.
        cos_sb = consts.tile([128, T], F32, tag="cos", name="cos")
        ss_sb = consts.tile([128, T], F32, tag="ss", name="ss")
        wk_sb = consts.tile([128, NKT * JG], F32R, tag="wk", name="wk")
        wv_sb = consts.tile([128, NKT * JG], F32R, tag="wv", name="wv")
        bq_sb = load_const("bq", bq_r, [1, JG])
        bk_sb = load_const("bk", bk_r, [1, JG])
        bv_sb = load_const("bv", bv_r, [1, JG])
        ones_sb = load_const("ones", ones_r, [1, 512])
        bm_sb = load_const("bmask", bmask, [128, 384])
        ind_sb = load_const("ind", ind, [2, JG])
        vones_sb = load_const("vones", vones, [128, HG])

        def _load_w_into(t, dram, width):
            nc.sync.dma_start(
                t[:].rearrange("p (i j) -> p i j", i=NKT),
                dram[:].bitcast(F32R).rearrange("(i p) j -> p i j", p=128),
            )

        def _load_w_ctile(t, dram, width, i):
            nc.sync.dma_start(
                t[:, i * width : (i + 1) * width],
                dram[128 * i : 128 * (i + 1), :].bitcast(F32R),
            )

        # (quarter, i) -> list of extra sync-queue loads to emit at that step
        deferred_loads = {
            (0, 5): [lambda: nc.sync.dma_start(cos_sb[:], cosT[:])],
            (0, 6): [lambda: nc.sync.dma_start(ss_sb[:], ssT[:])],
            (1, 0): [lambda: _load_w_into(wv_sb, wvT, JG)],
        }
        for _i in range(NKT):
            deferred_loads.setdefault((0, _i), []).insert(
                0, (lambda i=_i: (_load_w_ctile(wq_sb, wqT, JG, i),
                                  _load_w_ctile(wk_sb, wkT, JG, i)))
            )

        wp_sb = [None, None]
        for jt in range(2):
            wp_sb[jt] = consts.tile([128, C], F32R, tag=f"wp{jt}", name=f"wp{jt}")
            nc.gpsimd.dma_start(
                wp_sb[jt][:], wpT[128 * jt : 128 * (jt + 1), :].bitcast(F32R)
            )
        bp_sb = load_const("bp", bp_r, [1, C])

        qkv_sb = ctx.enter_context(tc.tile_pool(name="qkv", bufs=1))
        qT_sb = [qkv_sb.tile([128, T], F32R, tag=f"qT{j}", name=f"qT{j}") for j in range(2)]
        kT_sb = [qkv_sb.tile([128, T], F32R, tag=f"kT{j}", name=f"kT{j}") for j in range(2)]
        yT_sb = [qkv_sb.tile([128, T], F32R, tag=f"yT{j}", name=f"yT{j}") for j in range(2)]
        v_sb = [qkv_sb.tile([128, VW], F32R, tag=f"v{s}", name=f"v{s}") for s in range(NTT)]
        rr2_sb = [qkv_sb.tile([2, T], F32R, tag=f"rr2{j}", name=f"rr2{j}") for j in range(2)]
        rscr_sb = qkv_sb.tile([1, T], F32, tag="rscr")

        # ---- q/k phase: T-quarters, 4 slots (q-j0,k-j0,q-j1,k-j1) ------
        # Ropes pipeline under the next quarter's matmuls (psum 2 quarters
        # deep). Rotation = 4 partition-swapped ACT copies (pcr) + 3 wide
        # DVE ops.
        xstream = ctx.enter_context(tc.tile_pool(name="xstream", bufs=4))
        rope_pool = ctx.enter_context(tc.tile_pool(name="rope", bufs=3))
        with tc.tile_pool(name="pqk", bufs=8, space="PSUM") as pqk:
            for qtr in range(4):
                tlo = 512 * qtr
                ps4 = [
                    pqk.tile([128, 512], F32, tag="pqk", name="pqk")
                    for _ in range(4)
                ]
                for i in range(NKT):
                    for fn in deferred_loads.pop((qtr, i), []):
                        fn()
                    xt = xstream.tile([128, 512], F32R, tag="xq", name="xq")
                    nc.sync.dma_start(
                        xt[:],
                        xT[128 * i : 128 * (i + 1), tlo : tlo + 512].bitcast(F32R),
                    )
                    for sl, (jt, w_sb) in enumerate(
                        ((0, wq_sb), (0, wk_sb), (1, wq_sb), (1, wk_sb))
                    ):
                        nc.tensor.matmul(
                            ps4[sl][:],
                            w_sb[:, i * JG + 128 * jt : i * JG + 128 * (jt + 1)],
                            xt[:],
                            start=(i == 0),
                            stop=(zero_bias and i == NKT - 1),
                        )
                for sl, (jt, b_sb, dst) in enumerate(
                    (
                        (0, bq_sb, qT_sb),
                        (0, bk_sb, kT_sb),
                        (1, bq_sb, qT_sb),
                        (1, bk_sb, kT_sb),
                    )
                ):
                    p = ps4[sl]
                    if not zero_bias:
                        nc.tensor.matmul(
                            p[:],
                            b_sb[:, 128 * jt : 128 * (jt + 1)],
                            ones_sb[:, :],
                            start=False,
                            stop=True,
                        )
                    # RoPE: dst = p*cos + rotate_half(p)*ss
                    out = dst[jt][:, tlo : tlo + 512]
                    pcr = rope_pool.tile([128, 512], F32, tag="pcr", name="pcr")
                    for h0 in (0, 64):
                        a_, b_, c_ = h0, h0 + 32, h0 + 64
                        nc.scalar.copy(pcr[a_:b_, :], p[b_:c_, :])
                        nc.scalar.copy(pcr[b_:c_, :], p[a_:b_, :])
                    nc.vector.tensor_mul(out, p[:], cos_sb[:, tlo : tlo + 512])
                    rot = rope_pool.tile([128, 512], F32R, tag="rot", name="rot")
                    nc.vector.tensor_mul(rot[:], pcr[:], ss_sb[:, tlo : tlo + 512])
                    nc.vector.tensor_add(out, out, rot[:])

        # ---- v phase: natural layout, one accumulation group per bank --
        with tc.tile_pool(name="pv", bufs=4, space="PSUM") as pv:
            for tt in range(NTT):
                # column slab of xT for this t-tile: [128, 8 x 128]
                xc = xstream.tile([128, C], F32R, tag="xs", name="xs")
                nc.sync.dma_start(
                    xc[:].rearrange("p (i t) -> p i t", i=NKT),
                    xT[:, 128 * tt : 128 * (tt + 1)]
                    .bitcast(F32R)
                    .rearrange("(i p) t -> p i t", p=128),
                )

                p = pv.tile([128, JG], F32, tag="pv", name="pv")
                for i in range(NKT):
                    nc.tensor.matmul(
                        p[:],
                        xc[:, 128 * i : 128 * (i + 1)],
                        wv_sb[:, i * JG : (i + 1) * JG],
                        start=(i == 0),
                        stop=(zero_bias and i == NKT - 1),
                    )
                if not zero_bias:
                    nc.tensor.matmul(
                        p[:], ones_sb[:, :128], bv_sb[:, :], start=False, stop=True
                    )
                vv = v_sb[tt][:].rearrange("p (h w) -> p h w", h=HG)
                nc.gpsimd.tensor_copy(
                    vv[:, :, 64:65], vones_sb[:].rearrange("p (h w) -> p h w", w=1)
                )
                nc.scalar.copy(
                    vv[:, :, 0:64], p[:].rearrange("p (h w) -> p h w", h=HG)
                )

        # ---- attention: two T-half passes (pass A needs only half-0 rope)
        with (
            tc.tile_pool(name="pst", bufs=2, space="PSUM") as pst,
            tc.tile_pool(name="pav", bufs=1, space="PSUM") as pav,
            tc.tile_pool(name="prb", bufs=2, space="PSUM") as prb,
            tc.tile_pool(name="ppt", bufs=4) as ppt,
            tc.tile_pool(name="rb", bufs=2) as rb_pool,
        ):
            for lo in (0, 1024):
                nk = (lo + 1024) // 128  # s-tiles in this pass
                for hl in range(HG):
                    jt, m = hl // 2, hl % 2
                    qh = qT_sb[jt][64 * m : 64 * (m + 1), :]
                    kh = kT_sb[jt][64 * m : 64 * (m + 1), :]
                    p_av = pav.tile([65, 1024], F32, tag="pav", name="pav")
                    ppts = [None] * nk

                    def emit_st(k, ppts=ppts, qh=qh, kh=kh):
                        t0 = 128 * k
                        a = max(t0 - lo, 0)
                        estart = min(a, 256) if a < 512 else 512 + min(a - 512, 256)
                        p_st = pst.tile([128, 1024], F32, tag="pst", name="pst")
                        for s5 in range(2):
                            slo = lo + 512 * s5
                            if slo + 512 <= t0:
                                continue
                            sa = min(max(t0 - slo, 0), 256)
                            nc.tensor.matmul(
                                p_st[:, 512 * s5 + sa : 512 * (s5 + 1)],
                                kh[:, 128 * k : 128 * (k + 1)],
                                qh[:, slo + sa : slo + 512],
                                start=True,
                                stop=True,
                            )
                        pt = ppt.tile([128, 1024], F32R, tag="ppt", name="ppt")
                        nc.scalar.activation(
                            pt[:, estart:], p_st[:, estart:], EXP, scale=float(SCALE)
                        )
                        if t0 >= lo:
                            w = a + 128 - estart
                            nc.vector.tensor_mul(
                                pt[:, estart : a + 128],
                                pt[:, estart : a + 128],
                                bm_sb[:, 384 - w : 384],
                            )
                        ppts[k] = pt

                    def emit_av(k, ppts=ppts, p_av=p_av, hl=hl, nk=nk):
                        t0 = 128 * k
                        vh = v_sb[k][:, 65 * hl : 65 * (hl + 1)]
                        for cs in (lo // 512, lo // 512 + 1):
                            slo = 512 * cs
                            if slo + 512 <= t0:
                                continue
                            sa = min(max(t0 - slo, 0), 256)
                            off = slo - lo + sa
                            nc.tensor.matmul(
                                p_av[:, off : slo - lo + 512],
                                vh,
                                ppts[k][:, off : off + 512 - sa],
                                start=(k == 0),
                                stop=(k == min(4 * cs + 3, nk - 1)),
                            )

                    for k in range(nk):
                        emit_st(k)
                        if k > 0:
                            emit_av(k - 1)
                    emit_av(nk - 1)

                    nc.vector.tensor_copy(
                        yT_sb[jt][64 * m : 64 * (m + 1), lo : lo + 1024],
                        p_av[0:64, :],
                    )
                    nc.vector.tensor_copy(
                        rscr_sb[0:1, lo : lo + 1024].bitcast(F32R), p_av[64:65, :]
                    )
                    nc.sync.dma_start(
                        rr2_sb[jt][m : m + 1, lo : lo + 1024],
                        rscr_sb[0:1, lo : lo + 1024].bitcast(F32R),
                    )
                    if lo == 1024 and m == 1:
                        # head pair complete: normalize yT j-tile jt in place
                        for c4 in range(NC4):
                            pn = prb.tile([128, 512], F32, tag="prb", name="prb")
                            nc.tensor.matmul(
                                pn[:],
                                ind_sb[:, 128 * jt : 128 * (jt + 1)],
                                rr2_sb[jt][:, 512 * c4 : 512 * (c4 + 1)],
                                start=True,
                                stop=True,
                            )
                            rb = rb_pool.tile([128, 512], F32, tag="rb", name="rb")
                            nc.vector.reciprocal_approx_fast(out=rb[:], in_=pn[:])
                            sl = yT_sb[jt][:, 512 * c4 : 512 * (c4 + 1)]
                            nc.vector.tensor_mul(sl, sl, rb[:].bitcast(F32R))

        if debug:
            for j in range(2):
                nc.sync.dma_start(d_qT[j][:], qT_sb[j][:].bitcast(F32))
                nc.sync.dma_start(d_kT[j][:], kT_sb[j][:].bitcast(F32))
                nc.sync.dma_start(d_yT[j][:], yT_sb[j][:].bitcast(F32))
            for s in range(NTT):
                nc.sync.dma_start(d_v[s][:], v_sb[s][:].bitcast(F32))
            nc.sync.dma_start(d_rr4[:], rr4_sb[:].bitcast(F32))

        # ---- output projection (transposed, partial) -------------------
        with (
            tc.tile_pool(name="pp", bufs=4, space="PSUM") as pp,
            tc.tile_pool(name="ostage", bufs=2) as ostage,
        ):
            for et in range(8):
                o = ostage.tile([128, T], F32, tag="ostage")
                for c4 in range(NC4):
                    p = pp.tile([128, 512], F32, tag="pp")
                    for jt in range(2):
                        nc.tensor.matmul(
                            p[:],
                            wp_sb[jt][:, 128 * et : 128 * (et + 1)],
                            yT_sb[jt][:, 512 * c4 : 512 * (c4 + 1)],
                            start=(jt == 0),
                            stop=False,
                        )
                    nc.tensor.matmul(
                        p[:],
                        bp_sb[:, 128 * et : 128 * (et + 1)],
                        ones_sb[:, :],
                        start=False,
                        stop=True,
                    )
                    if c4 % 2 == 0:
                        nc.scalar.copy(o[:, 512 * c4 : 512 * (c4 + 1)], p[:])
                    else:
                        nc.vector.tensor_copy(o[:, 512 * c4 : 512 * (c4 + 1)], p[:])
                nc.sync.dma_start(outT[128 * et : 128 * (et + 1), :], o[:])

    nc.finalize()
    return nc


def _rope_tables():
    inv_freq = 1.0 / (10000.0 ** (np.arange(0, D, 2, dtype=np.float32) / D))
    t = np.arange(T, dtype=np.float32)
    freqs = t[:, None] * inv_freq[None, :]              # [T, 32]
    emb = np.concatenate([freqs, freqs], axis=1)        # [T, 64]
    cos = np.cos(emb).astype(np.float32).T              # [64, T]
    sin = np.sin(emb).astype(np.float32).T              # [64, T]
    # rotate_half signs at destination rows: rot[d<32] = -q[d+32]*sin[d]
    ss = np.concatenate([-sin[:32], sin[32:]], axis=0)
    cosT = np.concatenate([cos, cos], axis=0)           # [128, T] (2 heads)
    ssT = np.concatenate([ss, ss], axis=0)              # [128, T]
    return np.ascontiguousarray(cosT), np.ascontiguousarray(ssT)


def _host_inputs(x, Wq, bq, Wk, bk, Wv, bv, Wp, bp):
    cosT, ssT = _rope_tables()
    s = np.arange(128)[:, None]
    u = np.arange(384)[None, :]
    bmask = ((u - 256) >= s).astype(np.float32)
    ind = np.zeros((2, JG), np.float32)
    for j in range(JG):
        ind[(j // 64) % 2, j] = 1.0
    ones_r = np.ones((1, 512), np.float32)

    maps = []
    for b in range(B):
        for g in range(4):
            J = slice(g * JG, (g + 1) * JG)
            maps.append(
                {
                    "xT": np.ascontiguousarray(x[b].T),
                    "wqT": np.ascontiguousarray(Wq[J, :].T),
                    "wkT": np.ascontiguousarray(Wk[J, :].T),
                    "wvT": np.ascontiguousarray(Wv[J, :].T),
                    "wpT": np.ascontiguousarray(Wp[:, J].T),
                    "bq_r": bq[None, J].astype(np.float32),
                    "bk_r": bk[None, J].astype(np.float32),
                    "bv_r": bv[None, J].astype(np.float32),
                    "bp_r": (bp if g == 0 else np.zeros_like(bp))[None, :].astype(
                        np.float32
                    ),
                    "cosT": cosT,
                    "ssT": ssT,
                    "bmask": bmask,
                    "ind": ind,
                    "ones_r": ones_r,
                    "vones": np.ones((128, HG), np.float32),
                }
            )
    return maps


def kernel(x, Wq, bq, Wk, bk, Wv, bv, Wp, bp, _trace=False):
    global _NC_CACHE
    x, Wq, bq, Wk, bk, Wv, bv, Wp, bp = (
        np.asarray(a, np.float32) for a in (x, Wq, bq, Wk, bk, Wv, bv, Wp, bp)
    )
    zb = not (np.any(bq) or np.any(bk) or np.any(bv))
    if _NC_CACHE is None or _NC_CACHE[1] != zb:
        _NC_CACHE = (build_bass(zero_bias=zb), zb)
    maps = _host_inputs(x, Wq, bq, Wk, bk, Wv, bv, Wp, bp)
    res = run_bass_kernel_spmd(_NC_CACHE[0], maps, list(range(8)), trace=_trace)
    out = np.empty((B, T, C), np.float32)
    for b in range(B):
        acc = res.results[4 * b]["outT"].copy()
        for g in range(1, 4):
            acc += res.results[4 * b + g]["outT"]
        out[b] = acc.T
    if _trace:
        return out, res
    return out


# revision 21
# speedup vs baseline: 1.0138x; 1.0138x over previous
"""Causal self-attention with RoPE on 8 Trainium2 NeuronCores.

Sharding: core c = 4*b + g handles batch b (of 2) and head group g (4 of 16
heads). Each core computes q/k/v projections for its heads, head-local causal
softmax attention, and a partial output projection (Wp columns of its heads);
the host sums the 4 partials per batch.

Layout strategy (per core):
  xT    [C, T]  : x[b] transposed (host) — contraction operand for QKV.
  qT/kT [128,T] : per j-tile (2 heads each), partition = head dim.
  v''   [128,260] x16 : natural layout per t-tile; 65 cols/head =
                  [ones | v_head] so the AV matmul's row 0 accumulates the
                  softmax denominator for free.
  S^T   [s, t]  : scores transposed; exp(0.125*(S+mask)) on ACT -> P^T.
  AV    [65, t] : yT_unnorm (rows 1..64) + r (row 0) per head.
  norm  : 1/r broadcast to [128, t] via a K=4 indicator matmul; 1 DVE mult.
  proj  : outT[e, t] partial = WpT_g.T @ yT  (+ bp on group-leader core).

All matmul operands are float32r (TF32-like rounding, ~12 mantissa bits,
4x faster than fp32 on the PE). Causality is exploited at 128-tile
granularity; diagonal tiles are masked additively before the exp.
"""

import sys

for _p in ("/opt/trn_rl_repo",):
    if _p not in sys.path:
        sys.path.append(_p)

import numpy as np
from contextlib import ExitStack

import concourse.bacc as bacc
import concourse.tile as tile
from concourse import mybir
from concourse.bass_utils import run_bass_kernel_spmd

F32 = mybir.dt.float32
F32R = mybir.dt.float32r
EXP = mybir.ActivationFunctionType.Exp

B, T, C = 2, 2048, 1024
H, D = 16, 64
HG = 4                 # heads per core
JG = HG * D            # 256 j-columns per core
VW = HG * 65           # v'' width (ones col + 64 dims per head)
NKT = C // 128         # 8 contraction tiles
NTT = T // 128         # 16 t-tiles / s-tiles
NC4 = T // 512         # 4 512-chunks
MASK_VAL = -30000.0
SCALE = 1.0 / np.sqrt(D)

_NC_CACHE = None


def build_bass(debug=False, zero_bias=False):
    nc = bacc.Bacc()

    xT = nc.declare_dram_parameter("xT", [C, T], F32, isOutput=False)
    wqT = nc.declare_dram_parameter("wqT", [C, JG], F32, isOutput=False)
    wkT = nc.declare_dram_parameter("wkT", [C, JG], F32, isOutput=False)
    wvT = nc.declare_dram_parameter("wvT", [C, JG], F32, isOutput=False)
    wpT = nc.declare_dram_parameter("wpT", [JG, C], F32, isOutput=False)
    bq_r = nc.declare_dram_parameter("bq_r", [1, JG], F32, isOutput=False)
    bk_r = nc.declare_dram_parameter("bk_r", [1, JG], F32, isOutput=False)
    bv_r = nc.declare_dram_parameter("bv_r", [1, JG], F32, isOutput=False)
    bp_r = nc.declare_dram_parameter("bp_r", [1, C], F32, isOutput=False)
    cosT = nc.declare_dram_parameter("cosT", [128, T], F32, isOutput=False)
    ssT = nc.declare_dram_parameter("ssT", [128, T], F32, isOutput=False)
    bmask = nc.declare_dram_parameter("bmask", [128, 384], F32, isOutput=False)
    ind = nc.declare_dram_parameter("ind", [2, JG], F32, isOutput=False)
    ones_r = nc.declare_dram_parameter("ones_r", [1, 512], F32, isOutput=False)
    vones = nc.declare_dram_parameter("vones", [128, HG], F32, isOutput=False)

    outT = nc.declare_dram_parameter("outT", [C, T], F32, isOutput=True)
    if debug:
        d_qT = [nc.declare_dram_parameter(f"d_qT{j}", [128, T], F32, isOutput=True) for j in range(2)]
        d_kT = [nc.declare_dram_parameter(f"d_kT{j}", [128, T], F32, isOutput=True) for j in range(2)]
        d_yT = [nc.declare_dram_parameter(f"d_yT{j}", [128, T], F32, isOutput=True) for j in range(2)]
        d_v = [nc.declare_dram_parameter(f"d_v{s}", [128, VW], F32, isOutput=True) for s in range(NTT)]
        d_rr4 = nc.declare_dram_parameter("d_rr4", [HG, T], F32, isOutput=True)

    with (
        tile.TileContext(nc) as tc,
        ExitStack() as ctx,
        nc.allow_low_precision(reason="f32r matmul pipeline"),
    ):
        consts = ctx.enter_context(tc.tile_pool(name="consts", bufs=1))

        def load_const(name, dram, shape, dtype=F32R):
            t = consts.tile(shape, dtype, tag=name, name=name)
            src = dram[:] if dtype is F32 else dram[:].bitcast(F32R)
            nc.gpsimd.dma_start(t[:], src)
            return t

        # weights as [128, nk*width]: contraction tile i lives at cols [i*w,(i+1)*w)
        def load_w(name, dram, width, eng=None):
            t = consts.tile([128, NKT * width], F32R, tag=name, name=name)
            (eng or nc.gpsimd).dma_start(
                t[:].rearrange("p (i j) -> p i j", i=NKT),
                dram[:].bitcast(F32R).rearrange("(i p) j -> p i j", p=128),
            )
            return t

        wq_sb = consts.tile([128, NKT * JG], F32R, tag="wq", name="wq")
        # big consts are DMA'd on the sync queue interleaved with the xt
        # stream (see qk loop); small/late consts go via gpsimd SWDGE.
        cos_sb = consts.tile([128, T], F32, tag="cos", name="cos")
        ss_sb = consts.tile([128, T], F32, tag="ss", name="ss")
        wk_sb = consts.tile([128, NKT * JG], F32R, tag="wk", name="wk")
        wv_sb = consts.tile([128, NKT * JG], F32R, tag="wv", name="wv")
        bq_sb = load_const("bq", bq_r, [1, JG])
        bk_sb = load_const("bk", bk_r, [1, JG])
        bv_sb = load_const("bv", bv_r, [1, JG])
        ones_sb = load_const("ones", ones_r, [1, 512])
        bm_sb = load_const("bmask", bmask, [128, 384])
        ind_sb = load_const("ind", ind, [2, JG])
        vones_sb = load_const("vones", vones, [128, HG])

        def _load_w_into(t, dram, width):
            nc.sync.dma_start(
                t[:].rearrange("p (i j) -> p i j", i=NKT),
                dram[:].bitcast(F32R).rearrange("(i p) j -> p i j", p=128),
            )

        def _load_w_ctile(t, dram, width, i):
            nc.sync.dma_start(
                t[:, i * width : (i + 1) * width],
                dram[128 * i : 128 * (i + 1), :].bitcast(F32R),
            )

        # (quarter, i) -> list of extra sync-queue loads to emit at that step
        deferred_loads = {
            (0, 5): [lambda: nc.sync.dma_start(cos_sb[:], cosT[:])],
            (0, 6): [lambda: nc.sync.dma_start(ss_sb[:], ssT[:])],
            (1, 0): [lambda: _load_w_into(wv_sb, wvT, JG)],
        }
        for _i in range(NKT):
            deferred_loads.setdefault((0, _i), []).insert(
                0, (lambda i=_i: (_load_w_ctile(wq_sb, wqT, JG, i),
                                  _load_w_ctile(wk_sb, wkT, JG, i)))
            )

        wp_sb = [None, None]
        for jt in range(2):
            wp_sb[jt] = consts.tile([128, C], F32R, tag=f"wp{jt}", name=f"wp{jt}")
            nc.gpsimd.dma_start(
                wp_sb[jt][:], wpT[128 * jt : 128 * (jt + 1), :].bitcast(F32R)
            )
        bp_sb = load_const("bp", bp_r, [1, C])

        qkv_sb = ctx.enter_context(tc.tile_pool(name="qkv", bufs=1))
        qT_sb = [qkv_sb.tile([128, T], F32R, tag=f"qT{j}", name=f"qT{j}") for j in range(2)]
        kT_sb = [qkv_sb.tile([128, T], F32R, tag=f"kT{j}", name=f"kT{j}") for j in range(2)]
        yT_sb = [qkv_sb.tile([128, T], F32R, tag=f"yT{j}", name=f"yT{j}") for j in range(2)]
        v_sb = [qkv_sb.tile([128, VW], F32R, tag=f"v{s}", name=f"v{s}") for s in range(NTT)]
        rr2_sb = [qkv_sb.tile([2, T], F32R, tag=f"rr2{j}", name=f"rr2{j}") for j in range(2)]
        rscr_sb = qkv_sb.tile([1, T], F32, tag="rscr")

        # ---- q/k phase: T-quarters, 4 slots (q-j0,k-j0,q-j1,k-j1) ------
        # Ropes pipeline under the next quarter's matmuls (psum 2 quarters
        # deep). Rotation = 4 partition-swapped ACT copies (pcr) + 3 wide
        # DVE ops.
        xstream = ctx.enter_context(tc.tile_pool(name="xstream", bufs=4))
        rope_pool = ctx.enter_context(tc.tile_pool(name="rope", bufs=3))
        with tc.tile_pool(name="pqk", bufs=8, space="PSUM") as pqk:
            for qtr in range(4):
                tlo = 512 * qtr
                ps4 = [
                    pqk.tile([128, 512], F32, tag="pqk", name="pqk")
                    for _ in range(4)
                ]
                for i in range(NKT):
                    for fn in deferred_loads.pop((qtr, i), []):
                        fn()
                    xt = xstream.tile([128, 512], F32R, tag="xq", name="xq")
                    nc.sync.dma_start(
                        xt[:],
                        xT[128 * i : 128 * (i + 1), tlo : tlo + 512].bitcast(F32R),
                    )
                    for sl, (jt, w_sb) in enumerate(
                        ((0, wq_sb), (0, wk_sb), (1, wq_sb), (1, wk_sb))
                    ):
                        nc.tensor.matmul(
                            ps4[sl][:],
                            w_sb[:, i * JG + 128 * jt : i * JG + 128 * (jt + 1)],
                            xt[:],
                            start=(i == 0),
                            stop=(zero_bias and i == NKT - 1),
                        )
                for sl, (jt, b_sb, dst) in enumerate(
                    (
                        (0, bq_sb, qT_sb),
                        (0, bk_sb, kT_sb),
                        (1, bq_sb, qT_sb),
                        (1, bk_sb, kT_sb),
                    )
                ):
                    p = ps4[sl]
                    if not zero_bias:
                        nc.tensor.matmul(
                            p[:],
                            b_sb[:, 128 * jt : 128 * (jt + 1)],
                            ones_sb[:, :],
                            start=False,
                            stop=True,
                        )
                    # RoPE: dst = p*cos + rotate_half(p)*ss
                    out = dst[jt][:, tlo : tlo + 512]
                    pcr = rope_pool.tile([128, 512], F32, tag="pcr", name="pcr")
                    for h0 in (0, 64):
                        a_, b_, c_ = h0, h0 + 32, h0 + 64
                        nc.scalar.copy(pcr[a_:b_, :], p[b_:c_, :])
                        nc.scalar.copy(pcr[b_:c_, :], p[a_:b_, :])
                    nc.vector.tensor_mul(out, p[:], cos_sb[:, tlo : tlo + 512])
                    rot = rope_pool.tile([128, 512], F32R, tag="rot", name="rot")
                    nc.vector.tensor_mul(rot[:], pcr[:], ss_sb[:, tlo : tlo + 512])
                    nc.vector.tensor_add(out, out, rot[:])

        # ---- v phase: natural layout, one accumulation group per bank --
        with tc.tile_pool(name="pv", bufs=4, space="PSUM") as pv:
            for tt in range(NTT):
                # column slab of xT for this t-tile: [128, 8 x 128]
                xc = xstream.tile([128, C], F32R, tag="xs", name="xs")
                nc.sync.dma_start(
                    xc[:].rearrange("p (i t) -> p i t", i=NKT),
                    xT[:, 128 * tt : 128 * (tt + 1)]
                    .bitcast(F32R)
                    .rearrange("(i p) t -> p i t", p=128),
                )

                p = pv.tile([128, JG], F32, tag="pv", name="pv")
                for i in range(NKT):
                    nc.tensor.matmul(
                        p[:],
                        xc[:, 128 * i : 128 * (i + 1)],
                        wv_sb[:, i * JG : (i + 1) * JG],
                        start=(i == 0),
                        stop=(zero_bias and i == NKT - 1),
                    )
                if not zero_bias:
                    nc.tensor.matmul(
                        p[:], ones_sb[:, :128], bv_sb[:, :], start=False, stop=True
                    )
                vv = v_sb[tt][:].rearrange("p (h w) -> p h w", h=HG)
                nc.gpsimd.tensor_copy(
                    vv[:, :, 64:65], vones_sb[:].rearrange("p (h w) -> p h w", w=1)
                )
                nc.vector.tensor_copy(
                    vv[:, :, 0:64], p[:].rearrange("p (h w) -> p h w", h=HG)
                )

        # ---- attention: two T-half passes (pass A needs only half-0 rope)
        with (
            tc.tile_pool(name="pst", bufs=2, space="PSUM") as pst,
            tc.tile_pool(name="pav", bufs=1, space="PSUM") as pav,
            tc.tile_pool(name="prb", bufs=2, space="PSUM") as prb,
            tc.tile_pool(name="ppt", bufs=4) as ppt,
            tc.tile_pool(name="rb", bufs=2) as rb_pool,
        ):
            for lo in (0, 1024):
                nk = (lo + 1024) // 128  # s-tiles in this pass
                for hl in range(HG):
                    jt, m = hl // 2, hl % 2
                    qh = qT_sb[jt][64 * m : 64 * (m + 1), :]
                    kh = kT_sb[jt][64 * m : 64 * (m + 1), :]
                    p_av = pav.tile([65, 1024], F32, tag="pav", name="pav")
                    ppts = [None] * nk

                    def emit_st(k, ppts=ppts, qh=qh, kh=kh):
                        t0 = 128 * k
                        a = max(t0 - lo, 0)
                        estart = min(a, 256) if a < 512 else 512 + min(a - 512, 256)
                        p_st = pst.tile([128, 1024], F32, tag="pst", name="pst")
                        for s5 in range(2):
                            slo = lo + 512 * s5
                            if slo + 512 <= t0:
                                continue
                            sa = min(max(t0 - slo, 0), 256)
                            nc.tensor.matmul(
                                p_st[:, 512 * s5 + sa : 512 * (s5 + 1)],
                                kh[:, 128 * k : 128 * (k + 1)],
                                qh[:, slo + sa : slo + 512],
                                start=True,
                                stop=True,
                            )
                        pt = ppt.tile([128, 1024], F32R, tag="ppt", name="ppt")
                        nc.scalar.activation(
                            pt[:, estart:], p_st[:, estart:], EXP, scale=float(SCALE)
                        )
                        if t0 >= lo:
                            w = a + 128 - estart
                            nc.vector.tensor_mul(
                                pt[:, estart : a + 128],
                                pt[:, estart : a + 128],
                                bm_sb[:, 384 - w : 384],
                            )
                        ppts[k] = pt

                    def emit_av(k, ppts=ppts, p_av=p_av, hl=hl, nk=nk):
                        t0 = 128 * k
                        vh = v_sb[k][:, 65 * hl : 65 * (hl + 1)]
                        for cs in (lo // 512, lo // 512 + 1):
                            slo = 512 * cs
                            if slo + 512 <= t0:
                                continue
                            sa = min(max(t0 - slo, 0), 256)
                            off = slo - lo + sa
                            nc.tensor.matmul(
                                p_av[:, off : slo - lo + 512],
                                vh,
                                ppts[k][:, off : off + 512 - sa],
                                start=(k == 0),
                                stop=(k == min(4 * cs + 3, nk - 1)),
                            )

                    for k in range(nk):
                        emit_st(k)
                        if k > 0:
                            emit_av(k - 1)
                    emit_av(nk - 1)

                    nc.vector.tensor_copy(
                        yT_sb[jt][64 * m : 64 * (m + 1), lo : lo + 1024],
                        p_av[0:64, :],
                    )
                    nc.vector.tensor_copy(
                        rscr_sb[0:1, lo : lo + 1024].bitcast(F32R), p_av[64:65, :]
                    )
                    nc.sync.dma_start(
                        rr2_sb[jt][m : m + 1, lo : lo + 1024],
                        rscr_sb[0:1, lo : lo + 1024].bitcast(F32R),
                    )
                    if lo == 1024 and m == 1:
                        # head pair complete: normalize yT j-tile jt in place
                        for c4 in range(NC4):
                            pn = prb.tile([128, 512], F32, tag="prb", name="prb")
                            nc.tensor.matmul(
                                pn[:],
                                ind_sb[:, 128 * jt : 128 * (jt + 1)],
                                rr2_sb[jt][:, 512 * c4 : 512 * (c4 + 1)],
                                start=True,
                                stop=True,
                            )
                            rb = rb_pool.tile([128, 512], F32, tag="rb", name="rb")
                            nc.vector.reciprocal_approx_fast(out=rb[:], in_=pn[:])
                            sl = yT_sb[jt][:, 512 * c4 : 512 * (c4 + 1)]
                            nc.vector.tensor_mul(sl, sl, rb[:].bitcast(F32R))

        if debug:
            for j in range(2):
                nc.sync.dma_start(d_qT[j][:], qT_sb[j][:].bitcast(F32))
                nc.sync.dma_start(d_kT[j][:], kT_sb[j][:].bitcast(F32))
                nc.sync.dma_start(d_yT[j][:], yT_sb[j][:].bitcast(F32))
            for s in range(NTT):
                nc.sync.dma_start(d_v[s][:], v_sb[s][:].bitcast(F32))
            nc.sync.dma_start(d_rr4[:], rr4_sb[:].bitcast(F32))

        # ---- output projection (transposed, partial) -------------------
        with (
            tc.tile_pool(name="pp", bufs=4, space="PSUM") as pp,
            tc.tile_pool(name="ostage", bufs=2) as ostage,
        ):
            for et in range(8):
                o = ostage.tile([128, T], F32, tag="ostage")
                for c4 in range(NC4):
                    p = pp.tile([128, 512], F32, tag="pp")
                    for jt in range(2):
                        nc.tensor.matmul(
                            p[:],
                            wp_sb[jt][:, 128 * et : 128 * (et + 1)],
                            yT_sb[jt][:, 512 * c4 : 512 * (c4 + 1)],
                            start=(jt == 0),
                            stop=False,
                        )
                    nc.tensor.matmul(
                        p[:],
                        bp_sb[:, 128 * et : 128 * (et + 1)],
                        ones_sb[:, :],
                        start=False,
                        stop=True,
                    )
                    if c4 % 2 == 0:
                        nc.scalar.copy(o[:, 512 * c4 : 512 * (c4 + 1)], p[:])
                    else:
                        nc.vector.tensor_copy(o[:, 512 * c4 : 512 * (c4 + 1)], p[:])
                nc.sync.dma_start(outT[128 * et : 128 * (et + 1), :], o[:])

    nc.finalize()
    return nc


def _rope_tables():
    inv_freq = 1.0 / (10000.0 ** (np.arange(0, D, 2, dtype=np.float32) / D))
    t = np.arange(T, dtype=np.float32)
    freqs = t[:, None] * inv_freq[None, :]              # [T, 32]
    emb = np.concatenate([freqs, freqs], axis=1)        # [T, 64]
    cos = np.cos(emb).astype(np.float32).T              # [64, T]
    sin = np.sin(emb).astype(np.float32).T              # [64, T]
    # rotate_half signs at destination rows: rot[d<32] = -q[d+32]*sin[d]
    ss = np.concatenate([-sin[:32], sin[32:]], axis=0)
    cosT = np.concatenate([cos, cos], axis=0)           # [128, T] (2 heads)
    ssT = np.concatenate([ss, ss], axis=0)              # [128, T]
    return np.ascontiguousarray(cosT), np.ascontiguousarray(ssT)


def _host_inputs(x, Wq, bq, Wk, bk, Wv, bv, Wp, bp):
    cosT, ssT = _rope_tables()
    s = np.arange(128)[:, None]
    u = np.arange(384)[None, :]
    bmask = ((u - 256) >= s).astype(np.float32)
    ind = np.zeros((2, JG), np.float32)
    for j in range(JG):
        ind[(j // 64) % 2, j] = 1.0
    ones_r = np.ones((1, 512), np.float32)

    maps = []
    for b in range(B):
        for g in range(4):
            J = slice(g * JG, (g + 1) * JG)
            maps.append(
                {
                    "xT": np.ascontiguousarray(x[b].T),
                    "wqT": np.ascontiguousarray(Wq[J, :].T),
                    "wkT": np.ascontiguousarray(Wk[J, :].T),
                    "wvT": np.ascontiguousarray(Wv[J, :].T),
                    "wpT": np.ascontiguousarray(Wp[:, J].T),
                    "bq_r": bq[None, J].astype(np.float32),
                    "bk_r": bk[None, J].astype(np.float32),
                    "bv_r": bv[None, J].astype(np.float32),
                    "bp_r": (bp if g == 0 else np.zeros_like(bp))[None, :].astype(
                        np.float32
                    ),
                    "cosT": cosT,
                    "ssT": ssT,
                    "bmask": bmask,
                    "ind": ind,
                    "ones_r": ones_r,
                    "vones": np.ones((128, HG), np.float32),
                }
            )
    return maps


def kernel(x, Wq, bq, Wk, bk, Wv, bv, Wp, bp, _trace=False):
    global _NC_CACHE
    x, Wq, bq, Wk, bk, Wv, bv, Wp, bp = (
        np.asarray(a, np.float32) for a in (x, Wq, bq, Wk, bk, Wv, bv, Wp, bp)
    )
    zb = not (np.any(bq) or np.any(bk) or np.any(bv))
    if _NC_CACHE is None or _NC_CACHE[1] != zb:
        _NC_CACHE = (build_bass(zero_bias=zb), zb)
    maps = _host_inputs(x, Wq, bq, Wk, bk, Wv, bv, Wp, bp)
    res = run_bass_kernel_spmd(_NC_CACHE[0], maps, list(range(8)), trace=_trace)
    out = np.empty((B, T, C), np.float32)
    for b in range(B):
        acc = res.results[4 * b]["outT"].copy()
        for g in range(1, 4):
            acc += res.results[4 * b + g]["outT"]
        out[b] = acc.T
    if _trace:
        return out, res
    return out


# revision 22
# speedup vs baseline: 1.0923x; 1.0775x over previous
"""Causal self-attention with RoPE on 8 Trainium2 NeuronCores.

Sharding: core c = 4*b + g handles batch b (of 2) and head group g (4 of 16
heads). Each core computes q/k/v projections for its heads, head-local causal
softmax attention, and a partial output projection (Wp columns of its heads);
the host sums the 4 partials per batch.

Layout strategy (per core):
  xT    [C, T]  : x[b] transposed (host) — contraction operand for QKV.
  qT/kT [128,T] : per j-tile (2 heads each), partition = head dim.
  v''   [128,260] x16 : natural layout per t-tile; 65 cols/head =
                  [ones | v_head] so the AV matmul's row 0 accumulates the
                  softmax denominator for free.
  S^T   [s, t]  : scores transposed; exp(0.125*(S+mask)) on ACT -> P^T.
  AV    [65, t] : yT_unnorm (rows 1..64) + r (row 0) per head.
  norm  : 1/r broadcast to [128, t] via a K=4 indicator matmul; 1 DVE mult.
  proj  : outT[e, t] partial = WpT_g.T @ yT  (+ bp on group-leader core).

All matmul operands are float32r (TF32-like rounding, ~12 mantissa bits,
4x faster than fp32 on the PE). Causality is exploited at 128-tile
granularity; diagonal tiles are masked additively before the exp.
"""

import sys

for _p in ("/opt/trn_rl_repo",):
    if _p not in sys.path:
        sys.path.append(_p)

import numpy as np
from contextlib import ExitStack

import concourse.bacc as bacc
import concourse.tile as tile
from concourse import mybir
from concourse.bass_utils import run_bass_kernel_spmd

F32 = mybir.dt.float32
F32R = mybir.dt.float32r
EXP = mybir.ActivationFunctionType.Exp

B, T, C = 2, 2048, 1024
H, D = 16, 64
HG = 4                 # heads per core
JG = HG * D            # 256 j-columns per core
VW = HG * 65           # v'' width (ones col + 64 dims per head)
NKT = C // 128         # 8 contraction tiles
NTT = T // 128         # 16 t-tiles / s-tiles
NC4 = T // 512         # 4 512-chunks
MASK_VAL = -30000.0
SCALE = 1.0 / np.sqrt(D)

_NC_CACHE = None


def build_bass(debug=False, zero_bias=False):
    nc = bacc.Bacc()

    xT = nc.declare_dram_parameter("xT", [C, T], F32, isOutput=False)
    wqT = nc.declare_dram_parameter("wqT", [C, JG], F32, isOutput=False)
    wkT = nc.declare_dram_parameter("wkT", [C, JG], F32, isOutput=False)
    wvT = nc.declare_dram_parameter("wvT", [C, JG], F32, isOutput=False)
    wpT = nc.declare_dram_parameter("wpT", [JG, C], F32, isOutput=False)
    bq_r = nc.declare_dram_parameter("bq_r", [1, JG], F32, isOutput=False)
    bk_r = nc.declare_dram_parameter("bk_r", [1, JG], F32, isOutput=False)
    bv_r = nc.declare_dram_parameter("bv_r", [1, JG], F32, isOutput=False)
    bp_r = nc.declare_dram_parameter("bp_r", [1, C], F32, isOutput=False)
    cosT = nc.declare_dram_parameter("cosT", [128, T], F32, isOutput=False)
    ssT = nc.declare_dram_parameter("ssT", [128, T], F32, isOutput=False)
    bmask = nc.declare_dram_parameter("bmask", [128, 384], F32, isOutput=False)
    ind = nc.declare_dram_parameter("ind", [2, JG], F32, isOutput=False)
    ones_r = nc.declare_dram_parameter("ones_r", [1, 512], F32, isOutput=False)
    vones = nc.declare_dram_parameter("vones", [128, HG], F32, isOutput=False)

    outT = nc.declare_dram_parameter("outT", [C, T], F32, isOutput=True)
    if debug:
        d_qT = [nc.declare_dram_parameter(f"d_qT{j}", [128, T], F32, isOutput=True) for j in range(2)]
        d_kT = [nc.declare_dram_parameter(f"d_kT{j}", [128, T], F32, isOutput=True) for j in range(2)]
        d_yT = [nc.declare_dram_parameter(f"d_yT{j}", [128, T], F32, isOutput=True) for j in range(2)]
        d_v = [nc.declare_dram_parameter(f"d_v{s}", [128, VW], F32, isOutput=True) for s in range(NTT)]
        d_rr4 = nc.declare_dram_parameter("d_rr4", [HG, T], F32, isOutput=True)

    with (
        tile.TileContext(nc) as tc,
        ExitStack() as ctx,
        nc.allow_low_precision(reason="f32r matmul pipeline"),
    ):
        consts = ctx.enter_context(tc.tile_pool(name="consts", bufs=1))

        def load_const(name, dram, shape, dtype=F32R):
            t = consts.tile(shape, dtype, tag=name, name=name)
            src = dram[:] if dtype is F32 else dram[:].bitcast(F32R)
            nc.gpsimd.dma_start(t[:], src)
            return t

        # weights as [128, nk*width]: contraction tile i lives at cols [i*w,(i+1)*w)
        def load_w(name, dram, width, eng=None):
            t = consts.tile([128, NKT * width], F32R, tag=name, name=name)
            (eng or nc.gpsimd).dma_start(
                t[:].rearrange("p (i j) -> p i j", i=NKT),
                dram[:].bitcast(F32R).rearrange("(i p) j -> p i j", p=128),
            )
            return t

        wq_sb = consts.tile([128, NKT * JG], F32R, tag="wq", name="wq")
        # big consts are DMA'd on the sync queue interleaved with the xt
        # stream (see qk loop); small/late consts go via gpsimd SWDGE.
        cos_sb = consts.tile([128, T], F32, tag="cos", name="cos")
        ss_sb = consts.tile([128, T], F32, tag="ss", name="ss")
        wk_sb = consts.tile([128, NKT * JG], F32R, tag="wk", name="wk")
        wv_sb = consts.tile([128, NKT * JG], F32R, tag="wv", name="wv")
        bq_sb = load_const("bq", bq_r, [1, JG])
        bk_sb = load_const("bk", bk_r, [1, JG])
        bv_sb = load_const("bv", bv_r, [1, JG])
        ones_sb = load_const("ones", ones_r, [1, 512])
        bm_sb = load_const("bmask", bmask, [128, 384])
        ind_sb = load_const("ind", ind, [2, JG])
        vones_sb = load_const("vones", vones, [128, HG])

        def _load_w_into(t, dram, width):
            nc.sync.dma_start(
                t[:].rearrange("p (i j) -> p i j", i=NKT),
                dram[:].bitcast(F32R).rearrange("(i p) j -> p i j", p=128),
            )

        def _load_w_ctile(t, dram, width, i):
            nc.sync.dma_start(
                t[:, i * width : (i + 1) * width],
                dram[128 * i : 128 * (i + 1), :].bitcast(F32R),
            )

        # (quarter, i) -> list of extra sync-queue loads to emit at that step
        deferred_loads = {
            (0, 5): [lambda: nc.sync.dma_start(cos_sb[:], cosT[:])],
            (0, 6): [lambda: nc.sync.dma_start(ss_sb[:], ssT[:])],
            (1, 0): [lambda: _load_w_into(wv_sb, wvT, JG)],
        }
        for _i in range(NKT):
            deferred_loads.setdefault((0, _i), []).insert(
                0, (lambda i=_i: (_load_w_ctile(wq_sb, wqT, JG, i),
                                  _load_w_ctile(wk_sb, wkT, JG, i)))
            )

        wp_sb = [None, None]
        for jt in range(2):
            wp_sb[jt] = consts.tile([128, C], F32R, tag=f"wp{jt}", name=f"wp{jt}")
            nc.gpsimd.dma_start(
                wp_sb[jt][:], wpT[128 * jt : 128 * (jt + 1), :].bitcast(F32R)
            )
        bp_sb = load_const("bp", bp_r, [1, C])

        qkv_sb = ctx.enter_context(tc.tile_pool(name="qkv", bufs=1))
        qT_sb = [qkv_sb.tile([128, T], F32R, tag=f"qT{j}", name=f"qT{j}") for j in range(2)]
        kT_sb = [qkv_sb.tile([128, T], F32R, tag=f"kT{j}", name=f"kT{j}") for j in range(2)]
        yT_sb = [qkv_sb.tile([128, T], F32R, tag=f"yT{j}", name=f"yT{j}") for j in range(2)]
        v_sb = [qkv_sb.tile([128, VW], F32R, tag=f"v{s}", name=f"v{s}") for s in range(NTT)]
        rr2_sb = [qkv_sb.tile([2, T], F32R, tag=f"rr2{j}", name=f"rr2{j}") for j in range(2)]
        rscr_sb = qkv_sb.tile([1, T], F32, tag="rscr")

        # ---- q/k phase: T-quarters, 4 slots (q-j0,k-j0,q-j1,k-j1) ------
        # Ropes pipeline under the next quarter's matmuls (psum 2 quarters
        # deep). Rotation = 4 partition-swapped ACT copies (pcr) + 3 wide
        # DVE ops.
        xstream = ctx.enter_context(tc.tile_pool(name="xstream", bufs=4))
        rope_pool = ctx.enter_context(tc.tile_pool(name="rope", bufs=3))
        with tc.tile_pool(name="pqk", bufs=8, space="PSUM") as pqk:
            for qtr in range(4):
                tlo = 512 * qtr
                ps4 = [
                    pqk.tile([128, 512], F32, tag="pqk", name="pqk")
                    for _ in range(4)
                ]
                for i in range(NKT):
                    for fn in deferred_loads.pop((qtr, i), []):
                        fn()
                    xt = xstream.tile([128, 512], F32R, tag="xq", name="xq")
                    nc.sync.dma_start(
                        xt[:],
                        xT[128 * i : 128 * (i + 1), tlo : tlo + 512].bitcast(F32R),
                    )
                    for sl, (jt, w_sb) in enumerate(
                        ((0, wq_sb), (0, wk_sb), (1, wq_sb), (1, wk_sb))
                    ):
                        nc.tensor.matmul(
                            ps4[sl][:],
                            w_sb[:, i * JG + 128 * jt : i * JG + 128 * (jt + 1)],
                            xt[:],
                            start=(i == 0),
                            stop=(zero_bias and i == NKT - 1),
                        )
                for sl, (jt, b_sb, dst) in enumerate(
                    (
                        (0, bq_sb, qT_sb),
                        (0, bk_sb, kT_sb),
                        (1, bq_sb, qT_sb),
                        (1, bk_sb, kT_sb),
                    )
                ):
                    p = ps4[sl]
                    if not zero_bias:
                        nc.tensor.matmul(
                            p[:],
                            b_sb[:, 128 * jt : 128 * (jt + 1)],
                            ones_sb[:, :],
                            start=False,
                            stop=True,
                        )
                    # RoPE: dst = p*cos + rotate_half(p)*ss
                    out = dst[jt][:, tlo : tlo + 512]
                    pcr = rope_pool.tile([128, 512], F32, tag="pcr", name="pcr")
                    for h0 in (0, 64):
                        a_, b_, c_ = h0, h0 + 32, h0 + 64
                        nc.scalar.copy(pcr[a_:b_, :], p[b_:c_, :])
                        nc.scalar.copy(pcr[b_:c_, :], p[a_:b_, :])
                    nc.vector.tensor_mul(out, p[:], cos_sb[:, tlo : tlo + 512])
                    rot = rope_pool.tile([128, 512], F32R, tag="rot", name="rot")
                    nc.vector.tensor_mul(rot[:], pcr[:], ss_sb[:, tlo : tlo + 512])
                    nc.vector.tensor_add(out, out, rot[:])

        # ---- v phase: natural layout, one accumulation group per bank --
        with tc.tile_pool(name="pv", bufs=4, space="PSUM") as pv:
            for tt in range(NTT):
                # column slab of xT for this t-tile: [128, 8 x 128]
                xc = xstream.tile([128, C], F32R, tag="xs", name="xs")
                nc.sync.dma_start(
                    xc[:].rearrange("p (i t) -> p i t", i=NKT),
                    xT[:, 128 * tt : 128 * (tt + 1)]
                    .bitcast(F32R)
                    .rearrange("(i p) t -> p i t", p=128),
                )

                p = pv.tile([128, JG], F32, tag="pv", name="pv")
                for i in range(NKT):
                    nc.tensor.matmul(
                        p[:],
                        xc[:, 128 * i : 128 * (i + 1)],
                        wv_sb[:, i * JG : (i + 1) * JG],
                        start=(i == 0),
                        stop=(zero_bias and i == NKT - 1),
                    )
                if not zero_bias:
                    nc.tensor.matmul(
                        p[:], ones_sb[:, :128], bv_sb[:, :], start=False, stop=True
                    )
                vv = v_sb[tt][:].rearrange("p (h w) -> p h w", h=HG)
                nc.gpsimd.tensor_copy(
                    vv[:, :, 64:65], vones_sb[:].rearrange("p (h w) -> p h w", w=1)
                )
                nc.vector.tensor_copy(
                    vv[:, :, 0:64], p[:].rearrange("p (h w) -> p h w", h=HG)
                )

        # ---- attention: two T-half passes (pass A needs only half-0 rope)
        with (
            tc.tile_pool(name="pst", bufs=3, space="PSUM") as pst,
            tc.tile_pool(name="pav", bufs=1, space="PSUM") as pav,
            tc.tile_pool(name="ppt", bufs=4) as ppt,
        ):
            for lo in (0, 1024):
                nk = (lo + 1024) // 128  # s-tiles in this pass
                for hl in range(HG):
                    jt, m = hl // 2, hl % 2
                    qh = qT_sb[jt][64 * m : 64 * (m + 1), :]
                    kh = kT_sb[jt][64 * m : 64 * (m + 1), :]
                    p_av = pav.tile([65, 1024], F32, tag="pav", name="pav")
                    ppts = [None] * nk

                    def emit_st(k, ppts=ppts, qh=qh, kh=kh):
                        t0 = 128 * k
                        a = max(t0 - lo, 0)
                        estart = min(a, 256) if a < 512 else 512 + min(a - 512, 256)
                        p_st = pst.tile([128, 1024], F32, tag="pst", name="pst")
                        for s5 in range(2):
                            slo = lo + 512 * s5
                            if slo + 512 <= t0:
                                continue
                            sa = min(max(t0 - slo, 0), 256)
                            nc.tensor.matmul(
                                p_st[:, 512 * s5 + sa : 512 * (s5 + 1)],
                                kh[:, 128 * k : 128 * (k + 1)],
                                qh[:, slo + sa : slo + 512],
                                start=True,
                                stop=True,
                            )
                        pt = ppt.tile([128, 1024], F32R, tag="ppt", name="ppt")
                        nc.scalar.activation(
                            pt[:, estart:], p_st[:, estart:], EXP, scale=float(SCALE)
                        )
                        if t0 >= lo:
                            w = a + 128 - estart
                            nc.vector.tensor_mul(
                                pt[:, estart : a + 128],
                                pt[:, estart : a + 128],
                                bm_sb[:, 384 - w : 384],
                            )
                        ppts[k] = pt

                    def emit_av(k, ppts=ppts, p_av=p_av, hl=hl, nk=nk):
                        t0 = 128 * k
                        vh = v_sb[k][:, 65 * hl : 65 * (hl + 1)]
                        for cs in (lo // 512, lo // 512 + 1):
                            slo = 512 * cs
                            if slo + 512 <= t0:
                                continue
                            sa = min(max(t0 - slo, 0), 256)
                            off = slo - lo + sa
                            nc.tensor.matmul(
                                p_av[:, off : slo - lo + 512],
                                vh,
                                ppts[k][:, off : off + 512 - sa],
                                start=(k == 0),
                                stop=(k == min(4 * cs + 3, nk - 1)),
                            )

                    for k in range(nk):
                        emit_st(k)
                        if k > 0:
                            emit_av(k - 1)
                    emit_av(nk - 1)

                    nc.vector.tensor_copy(
                        yT_sb[jt][64 * m : 64 * (m + 1), lo : lo + 1024],
                        p_av[0:64, :],
                    )
                    nc.vector.tensor_copy(
                        rscr_sb[0:1, lo : lo + 1024].bitcast(F32R), p_av[64:65, :]
                    )
                    nc.sync.dma_start(
                        rr2_sb[jt][m : m + 1, lo : lo + 1024],
                        rscr_sb[0:1, lo : lo + 1024].bitcast(F32R),
                    )


        # ---- normalize (yT /= r per head) ------------------------------
        with (
            tc.tile_pool(name="prb", bufs=2, space="PSUM") as prb,
            tc.tile_pool(name="rb", bufs=2) as rb_pool,
        ):
            for jt in range(2):
                for c4 in range(NC4):
                    pn = prb.tile([128, 512], F32, tag="prb", name="prb")
                    nc.tensor.matmul(
                        pn[:],
                        ind_sb[:, 128 * jt : 128 * (jt + 1)],
                        rr2_sb[jt][:, 512 * c4 : 512 * (c4 + 1)],
                        start=True,
                        stop=True,
                    )
                    rb = rb_pool.tile([128, 512], F32, tag="rb", name="rb")
                    nc.vector.reciprocal_approx_fast(out=rb[:], in_=pn[:])
                    sl = yT_sb[jt][:, 512 * c4 : 512 * (c4 + 1)]
                    nc.vector.tensor_mul(sl, sl, rb[:].bitcast(F32R))

        if debug:
            for j in range(2):
                nc.sync.dma_start(d_qT[j][:], qT_sb[j][:].bitcast(F32))
                nc.sync.dma_start(d_kT[j][:], kT_sb[j][:].bitcast(F32))
                nc.sync.dma_start(d_yT[j][:], yT_sb[j][:].bitcast(F32))
            for s in range(NTT):
                nc.sync.dma_start(d_v[s][:], v_sb[s][:].bitcast(F32))
            nc.sync.dma_start(d_rr4[:], rr4_sb[:].bitcast(F32))

        # ---- output projection (transposed, partial) -------------------
        with (
            tc.tile_pool(name="pp", bufs=4, space="PSUM") as pp,
            tc.tile_pool(name="ostage", bufs=2) as ostage,
        ):
            for et in range(8):
                o = ostage.tile([128, T], F32, tag="ostage")
                for c4 in range(NC4):
                    p = pp.tile([128, 512], F32, tag="pp")
                    for jt in range(2):
                        nc.tensor.matmul(
                            p[:],
                            wp_sb[jt][:, 128 * et : 128 * (et + 1)],
                            yT_sb[jt][:, 512 * c4 : 512 * (c4 + 1)],
                            start=(jt == 0),
                            stop=False,
                        )
                    nc.tensor.matmul(
                        p[:],
                        bp_sb[:, 128 * et : 128 * (et + 1)],
                        ones_sb[:, :],
                        start=False,
                        stop=True,
                    )
                    if c4 % 2 == 0:
                        nc.scalar.copy(o[:, 512 * c4 : 512 * (c4 + 1)], p[:])
                    else:
                        nc.vector.tensor_copy(o[:, 512 * c4 : 512 * (c4 + 1)], p[:])
                nc.sync.dma_start(outT[128 * et : 128 * (et + 1), :], o[:])

    nc.finalize()
    return nc


def _rope_tables():
    inv_freq = 1.0 / (10000.0 ** (np.arange(0, D, 2, dtype=np.float32) / D))
    t = np.arange(T, dtype=np.float32)
    freqs = t[:, None] * inv_freq[None, :]              # [T, 32]
    emb = np.concatenate([freqs, freqs], axis=1)        # [T, 64]
    cos = np.cos(emb).astype(np.float32).T              # [64, T]
    sin = np.sin(emb).astype(np.float32).T              # [64, T]
    # rotate_half signs at destination rows: rot[d<32] = -q[d+32]*sin[d]
    ss = np.concatenate([-sin[:32], sin[32:]], axis=0)
    cosT = np.concatenate([cos, cos], axis=0)           # [128, T] (2 heads)
    ssT = np.concatenate([ss, ss], axis=0)              # [128, T]
    return np.ascontiguousarray(cosT), np.ascontiguousarray(ssT)


def _host_inputs(x, Wq, bq, Wk, bk, Wv, bv, Wp, bp):
    cosT, ssT = _rope_tables()
    s = np.arange(128)[:, None]
    u = np.arange(384)[None, :]
    bmask = ((u - 256) >= s).astype(np.float32)
    ind = np.zeros((2, JG), np.float32)
    for j in range(JG):
        ind[(j // 64) % 2, j] = 1.0
    ones_r = np.ones((1, 512), np.float32)

    maps = []
    for b in range(B):
        for g in range(4):
            J = slice(g * JG, (g + 1) * JG)
            maps.append(
                {
                    "xT": np.ascontiguousarray(x[b].T),
                    "wqT": np.ascontiguousarray(Wq[J, :].T),
                    "wkT": np.ascontiguousarray(Wk[J, :].T),
                    "wvT": np.ascontiguousarray(Wv[J, :].T),
                    "wpT": np.ascontiguousarray(Wp[:, J].T),
                    "bq_r": bq[None, J].astype(np.float32),
                    "bk_r": bk[None, J].astype(np.float32),
                    "bv_r": bv[None, J].astype(np.float32),
                    "bp_r": (bp if g == 0 else np.zeros_like(bp))[None, :].astype(
                        np.float32
                    ),
                    "cosT": cosT,
                    "ssT": ssT,
                    "bmask": bmask,
                    "ind": ind,
                    "ones_r": ones_r,
                    "vones": np.ones((128, HG), np.float32),
                }
            )
    return maps


def kernel(x, Wq, bq, Wk, bk, Wv, bv, Wp, bp, _trace=False):
    global _NC_CACHE
    x, Wq, bq, Wk, bk, Wv, bv, Wp, bp = (
        np.asarray(a, np.float32) for a in (x, Wq, bq, Wk, bk, Wv, bv, Wp, bp)
    )
    zb = not (np.any(bq) or np.any(bk) or np.any(bv))
    if _NC_CACHE is None or _NC_CACHE[1] != zb:
        _NC_CACHE = (build_bass(zero_bias=zb), zb)
    maps = _host_inputs(x, Wq, bq, Wk, bk, Wv, bv, Wp, bp)
    res = run_bass_kernel_spmd(_NC_CACHE[0], maps, list(range(8)), trace=_trace)
    out = np.empty((B, T, C), np.float32)
    for b in range(B):
        acc = res.results[4 * b]["outT"].copy()
        for g in range(1, 4):
            acc += res.results[4 * b + g]["outT"]
        out[b] = acc.T
    if _trace:
        return out, res
    return out
